# revision 10
# baseline (speedup 1.0000x reference)
"""2-layer GCN encoder (PyG GCNConv style) on 8 Trainium2 NeuronCores.

Strategy (node partitioning per the sharding hint):
- Nodes are partitioned into 8 contiguous shards (6250 per core); each core
  owns the aggregation for its shard's target nodes.
- Edges (with self-loops) are sorted by target and bucketed per core /
  per 128-target block; within a block they are split by source half
  (dma_gather indices are int16, so the 50k-row message tables are addressed
  as lo/hi halves) and sorted by source for HBM locality.
- Per core: h1 = (D^-1/2 x) @ W1 is computed redundantly on all cores
  (a full-x GEMM is cheaper than an AllGather of h1); per-edge messages are
  fetched with SWDGE dma_gather from the h1 table in HBM; the scatter-add is
  a PE matmul against an on-the-fly one-hot selector built on DVE
  (S[e, t] = (col_local[e] == t)); PSUM accumulates one 128-target block;
  the epilogue applies the target-side D^-1/2 + ReLU on ACT.
- relu(out1) shards are AllGathered (bf16), then layer 2 repeats the same
  structure with W2, reading transposed panels of the gathered activations.

The program is specialized to the input graph at run time: the edge schedule
(chunks per block) is compiled into the instruction stream, kept uniform
across cores (max over cores per block) so one SPMD program serves all 8.
"""

import glob
import sys

_b16 = sorted(glob.glob("/nix/store/*-b16-bazel-*/lib/python3.13/site-packages"))
if _b16 and _b16[-1] not in sys.path:
    sys.path.insert(0, _b16[-1])
if "/opt/trn_rl_repo" not in sys.path:
    sys.path.insert(1, "/opt/trn_rl_repo")

from dataclasses import dataclass

import ml_dtypes
import numpy as np

import concourse.bacc as bacc
import concourse.mybir as mybir
import concourse.tile as tile
from concourse.bass_utils import run_bass_kernel_spmd
from concourse.library_config import mlp

BF16 = mybir.dt.bfloat16
F32 = mybir.dt.float32
I16 = mybir.dt.int16
BF = ml_dtypes.bfloat16


@dataclass
class Cfg:
    n_nodes: int = 50000
    in_ch: int = 256
    hid: int = 128
    r: int = 8              # cores
    blk: int = 128          # targets per psum block
    chunk: int = 128        # edges per matmul chunk
    gemm_panel: int = 4096  # node columns per lhsT panel (GEMM1)

    @property
    def npc(self):
        return self.n_nodes // self.r

    @property
    def nblk(self):
        return -(-self.npc // self.blk)

    @property
    def pad_shard(self):
        return self.nblk * self.blk

    @property
    def split(self):
        return (self.r // 2) * self.npc

    @property
    def padf(self):
        return self.r * self.pad_shard

    @property
    def split2(self):
        return (self.r // 2) * self.pad_shard


def _wrap_idx(a):
    # logical i -> [i % 16, i // 16], replicated to 128 partitions
    a = np.asarray(a, np.int16)
    assert len(a) % 16 == 0
    return np.ascontiguousarray(np.tile(a.reshape(-1, 16).T, (8, 1)))


def _wrap_col(a):
    # chunk-major: edge j of chunk q -> [j, q]
    a = np.asarray(a, np.float32)
    assert len(a) % 128 == 0
    return np.ascontiguousarray(a.reshape(-1, 128).T.astype(BF))


def preprocess(edge_index, cfg: Cfg):
    """Host-side graph preprocessing -> (nch, per-core input arrays, dinv)."""
    N, R, NPC, BLK, NBLK, CH = (
        cfg.n_nodes, cfg.r, cfg.npc, cfg.blk, cfg.nblk, cfg.chunk,
    )
    ei = np.asarray(edge_index)
    loops = np.arange(N, dtype=np.int64)
    row = np.concatenate([ei[0].astype(np.int64), loops])
    col = np.concatenate([ei[1].astype(np.int64), loops])

    deg = np.bincount(col, minlength=N).astype(np.float64)
    dinv = np.where(deg > 0, 1.0 / np.sqrt(deg), 0.0).astype(np.float32)

    core = col // NPC
    blk = (col % NPC) // BLK
    hi = (row >= cfg.split).astype(np.int64)
    order = np.lexsort((row, hi, blk, core))
    row_s, col_s = row[order], col[order]
    core_s, blk_s, hi_s = core[order], blk[order], hi[order]

    key = (core_s * NBLK + blk_s) * 2 + hi_s
    counts = np.bincount(key, minlength=R * NBLK * 2).reshape(R, NBLK, 2)
    nch = np.maximum(-(-counts // CH), 1).max(axis=0)  # [NBLK, 2]

    seg_starts = np.zeros(R * NBLK * 2 + 1, np.int64)
    np.cumsum(counts.reshape(-1), out=seg_starts[1:])

    pad_off_mult = cfg.pad_shard - NPC  # pad rows inserted per rank

    per_core = []
    for c in range(R):
        arrs = {}
        for h in (0, 1):
            rows_list, cols_list = [], []
            for b in range(NBLK):
                k = (c * NBLK + b) * 2 + h
                s, e = seg_starts[k], seg_starts[k + 1]
                r_seg = row_s[s:e]
                c_seg = col_s[s:e] - c * NPC - b * BLK
                pad = nch[b, h] * CH - len(r_seg)
                pad_row = 0 if h == 0 else cfg.split
                rows_list += [r_seg, np.full(pad, pad_row, np.int64)]
                cols_list += [c_seg, np.full(pad, 255, np.int64)]
            rows = np.concatenate(rows_list)
            cols = np.concatenate(cols_list)
            rank = rows // NPC
            pad_rows = rows + rank * pad_off_mult
            i1 = rows if h == 0 else rows - cfg.split
            i2 = pad_rows if h == 0 else pad_rows - cfg.split2
            assert 0 <= i1.min() and i1.max() < 32768
            assert 0 <= i2.min() and i2.max() < 32768
            arrs[f"idx1{h}"] = _wrap_idx(i1)
            arrs[f"idx2{h}"] = _wrap_idx(i2)
            arrs[f"col{h}"] = _wrap_col(cols)

        dt = np.zeros((128, NBLK), np.float32)
        for b in range(NBLK):
            lo = c * NPC + b * BLK
            n = min(BLK, NPC - b * BLK)
            dt[:n, b] = dinv[lo:lo + n]
        arrs["dinv_t"] = dt
        per_core.append(arrs)

    dinv_pad = np.zeros(cfg.padf, np.float32)
    for r in range(R):
        dinv_pad[r * cfg.pad_shard: r * cfg.pad_shard + NPC] = (
            dinv[r * NPC: (r + 1) * NPC]
        )
    dinv2_w = np.ascontiguousarray(dinv_pad.reshape(-1, 128).T).astype(np.float32)
    for arrs in per_core:
        arrs["dinv2"] = dinv2_w

    return nch, per_core, dinv


def build_program(cfg: Cfg, nch, has_b1: bool, has_b2: bool):
    N, R, HID, IN_CH = cfg.n_nodes, cfg.r, cfg.hid, cfg.in_ch
    NBLK, BLK, CH = cfg.nblk, cfg.blk, cfg.chunk
    PAD, PADF = cfg.pad_shard, cfg.padf
    T = [int(nch[:, 0].sum()), int(nch[:, 1].sum())]
    loff = np.zeros((NBLK, 2), np.int64)
    loff[1:, 0] = np.cumsum(nch[:-1, 0])
    loff[1:, 1] = np.cumsum(nch[:-1, 1])

    nc = bacc.Bacc("TRN2", num_devices=R, num_swdge_queues=4)

    xT = nc.dram_tensor("xT", [IN_CH, N], BF16, kind="ExternalInput")
    w1 = nc.dram_tensor("W1", [IN_CH, HID], BF16, kind="ExternalInput")
    w2 = nc.dram_tensor("W2", [HID, HID], BF16, kind="ExternalInput")
    iota_in = nc.dram_tensor("iota", [128, 128], BF16, kind="ExternalInput")
    dinv_t_in = nc.dram_tensor("dinv_t", [128, NBLK], F32, kind="ExternalInput")
    dinv2_in = nc.dram_tensor("dinv2", [128, PADF // 128], F32,
                              kind="ExternalInput")
    idx_ins = {(l, h): nc.dram_tensor(f"idx{l}{h}", [128, T[h] * 8], I16,
                                      kind="ExternalInput")
               for l in (1, 2) for h in (0, 1)}
    col_ins = [nc.dram_tensor(f"col{h}", [128, T[h]], BF16,
                              kind="ExternalInput") for h in (0, 1)]
    b_ins = {}
    if has_b1:
        b_ins[1] = nc.dram_tensor("b1b", [128, HID], F32, kind="ExternalInput")
    if has_b2:
        b_ins[2] = nc.dram_tensor("b2b", [128, HID], F32, kind="ExternalInput")
    out = nc.dram_tensor("out", [cfg.npc, HID], F32, kind="ExternalOutput")

    h1p = nc.dram_tensor("h1p", [N, HID], BF16)
    h2p = nc.dram_tensor("h2p", [PADF, HID], BF16)
    r1s = nc.dram_tensor("r1s", [PAD, HID], BF16)
    r1f = nc.dram_tensor("r1f", [PADF, HID], BF16, addr_space="Shared")

    with tile.TileContext(nc) as tc:
        with (
            tc.tile_pool(name="const", bufs=1) as cpool,
            tc.tile_pool(name="idx", bufs=1) as ipool,
            tc.tile_pool(name="panel", bufs=2) as panpool,
            tc.tile_pool(name="gout", bufs=3) as gopool,
            tc.tile_pool(name="gather", bufs=3) as gapool,
            tc.tile_pool(name="stile", bufs=3) as spool,
            tc.tile_pool(name="epi", bufs=3) as epool,
            tc.tile_pool(name="psum", bufs=4, space="PSUM") as ppool,
        ):
            nc.gpsimd.load_library(mlp)

            iota_t = cpool.tile([128, 128], BF16)
            nc.sync.dma_start(iota_t[:], iota_in[:])
            dinv_t_t = cpool.tile([128, NBLK], F32)
            nc.sync.dma_start(dinv_t_t[:], dinv_t_in[:])
            dinv2_t = cpool.tile([128, PADF // 128], F32)
            nc.sync.dma_start(dinv2_t[:], dinv2_in[:])
            w1_t = cpool.tile([128, 2, HID], BF16)
            nc.sync.dma_start(w1_t[:, 0, :], w1[0:128, :])
            nc.sync.dma_start(w1_t[:, 1, :], w1[128:256, :])
            w2_t = cpool.tile([128, HID], BF16)
            nc.sync.dma_start(w2_t[:], w2[:])
            col_t = []
            for h in (0, 1):
                t = cpool.tile([128, T[h]], BF16, tag=f"colt{h}")
                nc.sync.dma_start(t[:], col_ins[h][:])
                col_t.append(t)
            b_t = {}
            for l, bi in b_ins.items():
                b_t[l] = cpool.tile([128, HID], F32, tag=f"bt{l}")
                nc.sync.dma_start(b_t[l][:], bi[:])

            def load_idx(layer):
                tiles = []
                for h in (0, 1):
                    t = ipool.tile([128, T[h] * 8], I16, tag=f"it{h}")
                    nc.sync.dma_start(t[:], idx_ins[(layer, h)][:])
                    tiles.append(t)
                return tiles

            def gemm(dst_dram, n_rows, layer):
                """dst = panel.T @ W (+ per-row D^-1/2 scale for layer 2)."""
                PANEL = cfg.gemm_panel if layer == 1 else min(2048, PAD)
                GRP = 8
                if layer == 1:
                    spans = [(p0, min(PANEL, n_rows - p0))
                             for p0 in range(0, n_rows, PANEL)]
                else:
                    spans = [(r * PAD + p0, min(PANEL, PAD - p0))
                             for r in range(R) for p0 in range(0, PAD, PANEL)]
                for p0, pn in spans:
                    if layer == 1:
                        pan = panpool.tile([128, 2, pn], BF16, tag="pan1")
                        nc.sync.dma_start(pan[:, 0, :], xT[0:128, p0:p0 + pn])
                        nc.sync.dma_start(pan[:, 1, :], xT[128:256, p0:p0 + pn])
                    else:
                        pan = panpool.tile([128, pn], BF16, tag="pan2")
                        nc.sync.dma_start(pan[:], r1f[p0:p0 + pn, :],
                                          transpose=True)
                    nchunks = -(-pn // 128)
                    for g0 in range(0, nchunks, GRP):
                        gn = min(GRP, nchunks - g0)
                        osb = gopool.tile([128, GRP, HID], BF16, tag="osb")
                        for j in range(g0, g0 + gn):
                            rn = min(128, pn - j * 128)
                            ps = ppool.tile([128, 128], F32, tag="gps")
                            if layer == 1:
                                nc.tensor.matmul(
                                    ps[:rn, :],
                                    lhsT=pan[:, 0, j * 128:j * 128 + rn],
                                    rhs=w1_t[:, 0, :], start=True, stop=False)
                                nc.tensor.matmul(
                                    ps[:rn, :],
                                    lhsT=pan[:, 1, j * 128:j * 128 + rn],
                                    rhs=w1_t[:, 1, :], start=False, stop=True)
                                nc.scalar.activation(
                                    osb[:rn, j - g0, :], ps[:rn, :],
                                    mybir.ActivationFunctionType.Copy)
                            else:
                                nc.tensor.matmul(
                                    ps[:rn, :],
                                    lhsT=pan[:, j * 128:j * 128 + rn],
                                    rhs=w2_t[:], start=True, stop=True)
                                gcol = (p0 + j * 128) // 128
                                nc.scalar.activation(
                                    osb[:rn, j - g0, :], ps[:rn, :],
                                    mybir.ActivationFunctionType.Copy,
                                    scale=dinv2_t[:, gcol:gcol + 1])
                        rows = min(gn * 128, pn - g0 * 128)
                        base = p0 + g0 * 128
                        nj = rows // 128
                        if nj:
                            nc.sync.dma_start(
                                dst_dram[base:base + nj * 128, :]
                                .rearrange("(j p) f -> p j f", p=128),
                                osb[:, 0:nj, :])
                        rem = rows - nj * 128
                        if rem:
                            nc.sync.dma_start(
                                dst_dram[base + nj * 128:base + rows, :],
                                osb[:rem, nj, :])

            qrr = [0]

            def agg(src_dram, idx_tiles, split_rows, dst_write):
                srcs = [src_dram[0:split_rows, :], src_dram[split_rows:, :]]
                for b in range(NBLK):
                    ps = ppool.tile([128, 128], F32, tag="aps")
                    tot = int(nch[b, 0] + nch[b, 1])
                    done = 0
                    for h in (0, 1):
                        n = int(nch[b, h])
                        if n == 0:
                            continue
                        off = int(loff[b, h])
                        dst = gapool.tile([128, n, HID], BF16, tag=f"gd{h}")
                        GCAP = 8  # chunks per dma_gather call
                        for s0 in range(0, n, GCAP):
                            sn = min(GCAP, n - s0)
                            nc.gpsimd.dma_gather(
                                dst[:, s0:s0 + sn, :], srcs[h],
                                idx_tiles[h][:, (off + s0) * 8:(off + s0 + sn) * 8],
                                sn * CH, sn * CH, HID,
                                queue_num=qrr[0] % 4)
                            qrr[0] += 1
                        S = spool.tile([128, n, 128], BF16, tag=f"st{h}")
                        nc.vector.tensor_tensor(
                            out=S[:],
                            in0=col_t[h][:, off:off + n].unsqueeze(2)
                                .to_broadcast([128, n, 128]),
                            in1=iota_t[:].unsqueeze(1)
                                .to_broadcast([128, n, 128]),
                            op=mybir.AluOpType.is_equal)
                        for q in range(n):
                            nc.tensor.matmul(
                                ps[:], lhsT=S[:, q, :], rhs=dst[:, q, :],
                                start=(done == 0), stop=(done == tot - 1))
                            done += 1
                    dst_write(b, ps)

            # ---- Phase 1: h1p = (D^-1/2 x) @ W1 (x pre-scaled on host) ----
            idx_l1 = load_idx(1)
            gemm(h1p, N, layer=1)

            # ---- Phase 2: layer-1 aggregation -> relu -> r1s ----
            def write1(b, ps):
                rsb = epool.tile([128, HID], BF16, tag="rsb")
                if not has_b1:
                    nc.scalar.activation(
                        rsb[:], ps[:], mybir.ActivationFunctionType.Relu,
                        scale=dinv_t_t[:, b:b + 1])
                else:
                    tmp = epool.tile([128, HID], F32, tag="tmp1")
                    nc.vector.tensor_scalar_mul(tmp[:], ps[:],
                                                dinv_t_t[:, b:b + 1])
                    nc.vector.tensor_tensor(out=tmp[:], in0=tmp[:],
                                            in1=b_t[1][:],
                                            op=mybir.AluOpType.add)
                    nc.scalar.activation(rsb[:], tmp[:],
                                         mybir.ActivationFunctionType.Relu)
                nc.sync.dma_start(r1s[b * BLK:(b + 1) * BLK, :], rsb[:])

            agg(h1p, idx_l1, cfg.split, write1)

            # ---- Phase 3: AllGather relu shards ----
            nc.gpsimd.collective_compute(
                "AllGather", mybir.AluOpType.bypass,
                replica_groups=[list(range(R))],
                ins=[r1s[:]], outs=[r1f[:]])

            # ---- Phase 4: h2p = D^-1/2 (r1f @ W2) ----
            idx_l2 = load_idx(2)
            gemm(h2p, PADF, layer=2)

            # ---- Phase 5: layer-2 aggregation -> out (f32) ----
            def write2(b, ps):
                osb2 = epool.tile([128, HID], F32, tag="osb2")
                nc.scalar.activation(
                    osb2[:], ps[:], mybir.ActivationFunctionType.Copy,
                    scale=dinv_t_t[:, b:b + 1])
                if has_b2:
                    nc.vector.tensor_tensor(out=osb2[:], in0=osb2[:],
                                            in1=b_t[2][:],
                                            op=mybir.AluOpType.add)
                rows = min(BLK, cfg.npc - b * BLK)
                nc.sync.dma_start(out[b * BLK:b * BLK + rows, :],
                                  osb2[:rows, :])

            agg(h2p, idx_l2, cfg.split2, write2)

    nc.compile()
    return nc


def make_in_maps(cfg: Cfg, per_core, x, dinv, W1, b1, W2, b2):
    xs = (np.asarray(x, np.float32) * dinv[:, None])
    xT = np.ascontiguousarray(xs.T).astype(BF)
    w1b = np.asarray(W1, np.float32).astype(BF)
    w2b = np.asarray(W2, np.float32).astype(BF)
    iota = np.tile(np.arange(128, dtype=np.float32), (128, 1)).astype(BF)
    has_b1 = bool(np.any(np.asarray(b1)))
    has_b2 = bool(np.any(np.asarray(b2)))
    in_maps = []
    for c in range(cfg.r):
        m = {"xT": xT, "W1": w1b, "W2": w2b, "iota": iota}
        m.update(per_core[c])
        if has_b1:
            m["b1b"] = np.tile(np.asarray(b1, np.float32), (128, 1))
        if has_b2:
            m["b2b"] = np.tile(np.asarray(b2, np.float32), (128, 1))
        in_maps.append(m)
    return in_maps, has_b1, has_b2


def kernel(x, edge_index, W1, b1, W2, b2):
    cfg = Cfg()
    nch, per_core, dinv = preprocess(edge_index, cfg)
    in_maps, has_b1, has_b2 = make_in_maps(cfg, per_core, x, dinv,
                                           W1, b1, W2, b2)
    nc = build_program(cfg, nch, has_b1, has_b2)
    res = run_bass_kernel_spmd(nc, in_maps, list(range(cfg.r)))
    return np.concatenate([res.results[c]["out"] for c in range(cfg.r)],
                          axis=0)


# revision 14
# speedup vs baseline: 1.0168x; 1.0168x over previous
"""2-layer GCN encoder (PyG GCNConv style) on 8 Trainium2 NeuronCores.

Strategy (node partitioning per the sharding hint):
- Nodes are partitioned into 8 contiguous shards (6250 per core); each core
  owns the aggregation for its shard's target nodes.
- Edges (with self-loops) are sorted by target and bucketed per core /
  per 128-target block; within a block they are split by source half
  (dma_gather indices are int16, so the 50k-row message tables are addressed
  as lo/hi halves) and sorted by source for HBM locality.
- Per core: h1 = (D^-1/2 x) @ W1 is computed redundantly on all cores
  (a full-x GEMM is cheaper than an AllGather of h1); per-edge messages are
  fetched with SWDGE dma_gather from the h1 table in HBM; the scatter-add is
  a PE matmul against an on-the-fly one-hot selector built on DVE
  (S[e, t] = (col_local[e] == t)); PSUM accumulates one 128-target block;
  the epilogue applies the target-side D^-1/2 + ReLU on ACT.
- relu(out1) shards are AllGathered (bf16), then layer 2 repeats the same
  structure with W2, reading transposed panels of the gathered activations.

The program is specialized to the input graph at run time: the edge schedule
(chunks per block) is compiled into the instruction stream, kept uniform
across cores (max over cores per block) so one SPMD program serves all 8.
"""

import glob
import sys

_b16 = sorted(glob.glob("/nix/store/*-b16-bazel-*/lib/python3.13/site-packages"))
if _b16 and _b16[-1] not in sys.path:
    sys.path.insert(0, _b16[-1])
if "/opt/trn_rl_repo" not in sys.path:
    sys.path.insert(1, "/opt/trn_rl_repo")

from dataclasses import dataclass

import ml_dtypes
import numpy as np

import concourse.bacc as bacc
import concourse.mybir as mybir
import concourse.tile as tile
from concourse.bass_utils import run_bass_kernel_spmd
from concourse.library_config import mlp

BF16 = mybir.dt.bfloat16
F32 = mybir.dt.float32
I16 = mybir.dt.int16
BF = ml_dtypes.bfloat16


@dataclass
class Cfg:
    n_nodes: int = 50000
    in_ch: int = 256
    hid: int = 128
    r: int = 8              # cores
    blk: int = 128          # targets per psum block
    chunk: int = 128        # edges per matmul chunk
    gemm_panel: int = 4096  # node columns per lhsT panel (GEMM1)

    @property
    def npc(self):
        return self.n_nodes // self.r

    @property
    def nblk(self):
        return -(-self.npc // self.blk)

    @property
    def pad_shard(self):
        return self.nblk * self.blk

    @property
    def split(self):
        return (self.r // 2) * self.npc

    @property
    def padf(self):
        return self.r * self.pad_shard

    @property
    def split2(self):
        return (self.r // 2) * self.pad_shard


def _wrap_idx(a):
    # logical i -> [i % 16, i // 16], replicated to 128 partitions
    a = np.asarray(a, np.int16)
    assert len(a) % 16 == 0
    return np.ascontiguousarray(np.tile(a.reshape(-1, 16).T, (8, 1)))


def _wrap_col(a):
    # chunk-major: edge j of chunk q -> [j, q]
    a = np.asarray(a, np.float32)
    assert len(a) % 128 == 0
    return np.ascontiguousarray(a.reshape(-1, 128).T.astype(BF))


def preprocess(edge_index, cfg: Cfg):
    """Host-side graph preprocessing -> (nch, per-core input arrays, dinv)."""
    N, R, NPC, BLK, NBLK, CH = (
        cfg.n_nodes, cfg.r, cfg.npc, cfg.blk, cfg.nblk, cfg.chunk,
    )
    ei = np.asarray(edge_index)
    loops = np.arange(N, dtype=np.int64)
    row = np.concatenate([ei[0].astype(np.int64), loops])
    col = np.concatenate([ei[1].astype(np.int64), loops])

    deg = np.bincount(col, minlength=N).astype(np.float64)
    dinv = np.where(deg > 0, 1.0 / np.sqrt(deg), 0.0).astype(np.float32)

    core = col // NPC
    blk = (col % NPC) // BLK
    hi = (row >= cfg.split).astype(np.int64)
    order = np.lexsort((row, hi, blk, core))
    row_s, col_s = row[order], col[order]
    core_s, blk_s, hi_s = core[order], blk[order], hi[order]

    key = (core_s * NBLK + blk_s) * 2 + hi_s
    counts = np.bincount(key, minlength=R * NBLK * 2).reshape(R, NBLK, 2)
    nch = np.maximum(-(-counts // CH), 1).max(axis=0)  # [NBLK, 2]

    seg_starts = np.zeros(R * NBLK * 2 + 1, np.int64)
    np.cumsum(counts.reshape(-1), out=seg_starts[1:])

    pad_off_mult = cfg.pad_shard - NPC  # pad rows inserted per rank

    per_core = []
    for c in range(R):
        arrs = {}
        for h in (0, 1):
            rows_list, cols_list = [], []
            for b in range(NBLK):
                k = (c * NBLK + b) * 2 + h
                s, e = seg_starts[k], seg_starts[k + 1]
                r_seg = row_s[s:e]
                c_seg = col_s[s:e] - c * NPC - b * BLK
                pad = nch[b, h] * CH - len(r_seg)
                pad_row = 0 if h == 0 else cfg.split
                rows_list += [r_seg, np.full(pad, pad_row, np.int64)]
                cols_list += [c_seg, np.full(pad, 255, np.int64)]
            rows = np.concatenate(rows_list)
            cols = np.concatenate(cols_list)
            rank = rows // NPC
            pad_rows = rows + rank * pad_off_mult
            i1 = rows if h == 0 else rows - cfg.split
            i2 = pad_rows if h == 0 else pad_rows - cfg.split2
            assert 0 <= i1.min() and i1.max() < 32768
            assert 0 <= i2.min() and i2.max() < 32768
            arrs[f"idx1{h}"] = _wrap_idx(i1)
            arrs[f"idx2{h}"] = _wrap_idx(i2)
            arrs[f"col{h}"] = _wrap_col(cols)

        dt = np.zeros((128, NBLK), np.float32)
        for b in range(NBLK):
            lo = c * NPC + b * BLK
            n = min(BLK, NPC - b * BLK)
            dt[:n, b] = dinv[lo:lo + n]
        arrs["dinv_t"] = dt
        arrs["dinv_tsq"] = dt * dt
        per_core.append(arrs)

    return nch, per_core, dinv


def build_program(cfg: Cfg, nch, has_b1: bool, has_b2: bool):
    N, R, HID, IN_CH = cfg.n_nodes, cfg.r, cfg.hid, cfg.in_ch
    NBLK, BLK, CH = cfg.nblk, cfg.blk, cfg.chunk
    PAD, PADF = cfg.pad_shard, cfg.padf
    T = [int(nch[:, 0].sum()), int(nch[:, 1].sum())]
    loff = np.zeros((NBLK, 2), np.int64)
    loff[1:, 0] = np.cumsum(nch[:-1, 0])
    loff[1:, 1] = np.cumsum(nch[:-1, 1])

    nc = bacc.Bacc("TRN2", num_devices=R, num_swdge_queues=4)

    xT = nc.dram_tensor("xT", [IN_CH, N], BF16, kind="ExternalInput")
    w1 = nc.dram_tensor("W1", [IN_CH, HID], BF16, kind="ExternalInput")
    w2 = nc.dram_tensor("W2", [HID, HID], BF16, kind="ExternalInput")
    iota_in = nc.dram_tensor("iota", [128, 128], BF16, kind="ExternalInput")
    dinv_t_in = nc.dram_tensor("dinv_t", [128, NBLK], F32, kind="ExternalInput")
    dinv_tsq_in = nc.dram_tensor("dinv_tsq", [128, NBLK], F32,
                                 kind="ExternalInput")
    idx_ins = {(l, h): nc.dram_tensor(f"idx{l}{h}", [128, T[h] * 8], I16,
                                      kind="ExternalInput")
               for l in (1, 2) for h in (0, 1)}
    col_ins = [nc.dram_tensor(f"col{h}", [128, T[h]], BF16,
                              kind="ExternalInput") for h in (0, 1)]
    b_ins = {}
    if has_b1:
        b_ins[1] = nc.dram_tensor("b1b", [128, HID], F32, kind="ExternalInput")
    if has_b2:
        b_ins[2] = nc.dram_tensor("b2b", [128, HID], F32, kind="ExternalInput")
    out = nc.dram_tensor("out", [cfg.npc, HID], F32, kind="ExternalOutput")

    h1p = nc.dram_tensor("h1p", [N, HID], BF16)
    h2lo = nc.dram_tensor("h2lo", [PADF // 2, HID], BF16)
    h2hi = nc.dram_tensor("h2hi", [PADF // 2, HID], BF16)
    r1s = nc.dram_tensor("r1s", [PAD, HID], BF16)
    r1f = nc.dram_tensor("r1f", [PADF, HID], BF16, addr_space="Shared")

    with tile.TileContext(nc) as tc:
        with (
            tc.tile_pool(name="const", bufs=1) as cpool,
            tc.tile_pool(name="idx", bufs=1) as ipool,
            tc.tile_pool(name="panel", bufs=2) as panpool,
            tc.tile_pool(name="gout", bufs=3) as gopool,
            tc.tile_pool(name="gather", bufs=3) as gapool,
            tc.tile_pool(name="stile", bufs=3) as spool,
            tc.tile_pool(name="epi", bufs=3) as epool,
            tc.tile_pool(name="psum", bufs=4, space="PSUM") as ppool,
        ):
            nc.gpsimd.load_library(mlp)

            iota_t = cpool.tile([128, 128], BF16)
            nc.sync.dma_start(iota_t[:], iota_in[:])
            dinv_t_t = cpool.tile([128, NBLK], F32)
            nc.sync.dma_start(dinv_t_t[:], dinv_t_in[:])
            dinv_tsq_t = cpool.tile([128, NBLK], F32)
            nc.sync.dma_start(dinv_tsq_t[:], dinv_tsq_in[:])
            w1_t = cpool.tile([128, 2, HID], BF16)
            nc.sync.dma_start(w1_t[:, 0, :], w1[0:128, :])
            nc.sync.dma_start(w1_t[:, 1, :], w1[128:256, :])
            w2_t = cpool.tile([128, HID], BF16)
            nc.sync.dma_start(w2_t[:], w2[:])
            col_t = []
            for h in (0, 1):
                t = cpool.tile([128, T[h]], BF16, tag=f"colt{h}")
                nc.sync.dma_start(t[:], col_ins[h][:])
                col_t.append(t)
            b_t = {}
            for l, bi in b_ins.items():
                b_t[l] = cpool.tile([128, HID], F32, tag=f"bt{l}")
                nc.sync.dma_start(b_t[l][:], bi[:])

            def load_idx(layer):
                tiles = []
                for h in (0, 1):
                    t = ipool.tile([128, T[h] * 8], I16, tag=f"it{h}")
                    nc.sync.dma_start(t[:], idx_ins[(layer, h)][:])
                    tiles.append(t)
                return tiles

            def gemm(layer):
                """dst = panel.T @ W, batched PSUM-bank epilogues."""
                PANEL = cfg.gemm_panel if layer == 1 else min(2048, PAD)
                GRP = 8   # chunks per output DMA
                PSG = 4   # chunks per psum bank
                if layer == 1:
                    spans = [(h1p, p0, p0, min(PANEL, N - p0))
                             for p0 in range(0, N, PANEL)]
                else:
                    spans = []
                    for r in range(R):
                        dst = h2lo if r < R // 2 else h2hi
                        for p0 in range(0, PAD, PANEL):
                            spans.append((dst, (r % (R // 2)) * PAD + p0,
                                          r * PAD + p0, min(PANEL, PAD - p0)))
                for dst_dram, dbase, sbase, pn in spans:
                    if layer == 1:
                        pan = panpool.tile([128, 2, pn], BF16, tag="pan1")
                        nc.sync.dma_start(pan[:, 0, :], xT[0:128, sbase:sbase + pn])
                        nc.sync.dma_start(pan[:, 1, :], xT[128:256, sbase:sbase + pn])
                    else:
                        pan = panpool.tile([128, pn], BF16, tag="pan2")
                        nc.sync.dma_start(pan[:], r1f[sbase:sbase + pn, :],
                                          transpose=True)
                    nchunks = -(-pn // 128)
                    for g0 in range(0, nchunks, GRP):
                        gn = min(GRP, nchunks - g0)
                        osb = gopool.tile([128, GRP, HID], BF16, tag="osb")
                        for q0 in range(g0, g0 + gn, PSG):
                            qn = min(PSG, g0 + gn - q0)
                            ps = ppool.tile([128, PSG * 128], F32, tag="gps")
                            full = (pn - q0 * 128) >= qn * 128
                            for j in range(q0, q0 + qn):
                                rn = min(128, pn - j * 128)
                                w = (j - q0) * 128
                                if layer == 1:
                                    nc.tensor.matmul(
                                        ps[:rn, w:w + 128],
                                        lhsT=pan[:, 0, j * 128:j * 128 + rn],
                                        rhs=w1_t[:, 0, :], start=True, stop=False)
                                    nc.tensor.matmul(
                                        ps[:rn, w:w + 128],
                                        lhsT=pan[:, 1, j * 128:j * 128 + rn],
                                        rhs=w1_t[:, 1, :], start=False, stop=True)
                                else:
                                    nc.tensor.matmul(
                                        ps[:rn, w:w + 128],
                                        lhsT=pan[:, j * 128:j * 128 + rn],
                                        rhs=w2_t[:], start=True, stop=True)
                            if full:
                                nc.scalar.activation(
                                    osb[:, q0 - g0:q0 - g0 + qn, :],
                                    ps[:, :qn * 128]
                                    .rearrange("p (j f) -> p j f", f=HID),
                                    mybir.ActivationFunctionType.Copy)
                            else:
                                for j in range(q0, q0 + qn):
                                    rn = min(128, pn - j * 128)
                                    w = (j - q0) * 128
                                    nc.scalar.activation(
                                        osb[:rn, j - g0, :], ps[:rn, w:w + 128],
                                        mybir.ActivationFunctionType.Copy)
                        rows = min(gn * 128, pn - g0 * 128)
                        base = dbase + g0 * 128
                        nj = rows // 128
                        if nj:
                            nc.sync.dma_start(
                                dst_dram[base:base + nj * 128, :]
                                .rearrange("(j p) f -> p j f", p=128),
                                osb[:, 0:nj, :])
                        rem = rows - nj * 128
                        if rem:
                            nc.sync.dma_start(
                                dst_dram[base + nj * 128:base + rows, :],
                                osb[:rem, nj, :])

            qrr = [0]

            def agg(srcs, idx_tiles, dst_write):
                for b in range(NBLK):
                    ps = ppool.tile([128, 128], F32, tag="aps")
                    tot = int(nch[b, 0] + nch[b, 1])
                    done = 0
                    for h in (0, 1):
                        n = int(nch[b, h])
                        if n == 0:
                            continue
                        off = int(loff[b, h])
                        dst = gapool.tile([128, n, HID], BF16, tag=f"gd{h}")
                        GCAP = 8  # chunks per dma_gather call
                        for s0 in range(0, n, GCAP):
                            sn = min(GCAP, n - s0)
                            nc.gpsimd.dma_gather(
                                dst[:, s0:s0 + sn, :], srcs[h],
                                idx_tiles[h][:, (off + s0) * 8:(off + s0 + sn) * 8],
                                sn * CH, sn * CH, HID,
                                queue_num=qrr[0] % 4)
                            qrr[0] += 1
                        S = spool.tile([128, n, 128], BF16, tag=f"st{h}")
                        nc.vector.tensor_tensor(
                            out=S[:],
                            in0=col_t[h][:, off:off + n].unsqueeze(2)
                                .to_broadcast([128, n, 128]),
                            in1=iota_t[:].unsqueeze(1)
                                .to_broadcast([128, n, 128]),
                            op=mybir.AluOpType.is_equal)
                        for q in range(n):
                            nc.tensor.matmul(
                                ps[:], lhsT=S[:, q, :], rhs=dst[:, q, :],
                                start=(done == 0), stop=(done == tot - 1))
                            done += 1
                    dst_write(b, ps)

            # ---- Phase 1: h1p = (D^-1/2 x) @ W1 (x pre-scaled on host) ----
            idx_l1 = load_idx(1)
            gemm(layer=1)

            # ---- Phase 2: layer-1 aggregation -> relu -> r1s ----
            def write1(b, ps):
                rsb = epool.tile([128, HID], BF16, tag="rsb")
                if not has_b1:
                    nc.scalar.activation(
                        rsb[:], ps[:], mybir.ActivationFunctionType.Relu,
                        scale=dinv_tsq_t[:, b:b + 1])
                else:
                    tmp = epool.tile([128, HID], F32, tag="tmp1")
                    nc.vector.tensor_scalar_mul(tmp[:], ps[:],
                                                dinv_t_t[:, b:b + 1])
                    nc.vector.tensor_tensor(out=tmp[:], in0=tmp[:],
                                            in1=b_t[1][:],
                                            op=mybir.AluOpType.add)
                    # dinv * relu(y) == relu(dinv * y) for dinv > 0
                    nc.scalar.activation(rsb[:], tmp[:],
                                         mybir.ActivationFunctionType.Relu,
                                         scale=dinv_t_t[:, b:b + 1])
                nc.sync.dma_start(r1s[b * BLK:(b + 1) * BLK, :], rsb[:])

            agg([h1p[0:cfg.split, :], h1p[cfg.split:, :]], idx_l1, write1)

            # ---- Phase 3: AllGather relu shards ----
            nc.gpsimd.collective_compute(
                "AllGather", mybir.AluOpType.bypass,
                replica_groups=[list(range(R))],
                ins=[r1s[:]], outs=[r1f[:]])

            # ---- Phase 4: h2p = D^-1/2 (r1f @ W2) ----
            idx_l2 = load_idx(2)
            gemm(layer=2)

            # ---- Phase 5: layer-2 aggregation -> out (f32) ----
            def write2(b, ps):
                osb2 = epool.tile([128, HID], F32, tag="osb2")
                nc.scalar.activation(
                    osb2[:], ps[:], mybir.ActivationFunctionType.Copy,
                    scale=dinv_t_t[:, b:b + 1])
                if has_b2:
                    nc.vector.tensor_tensor(out=osb2[:], in0=osb2[:],
                                            in1=b_t[2][:],
                                            op=mybir.AluOpType.add)
                rows = min(BLK, cfg.npc - b * BLK)
                nc.sync.dma_start(out[b * BLK:b * BLK + rows, :],
                                  osb2[:rows, :])

            agg([h2lo[:], h2hi[:]], idx_l2, write2)

    nc.compile()
    return nc


def make_in_maps(cfg: Cfg, per_core, x, dinv, W1, b1, W2, b2):
    xs = (np.asarray(x, np.float32) * dinv[:, None])
    xT = np.ascontiguousarray(xs.T).astype(BF)
    w1b = np.asarray(W1, np.float32).astype(BF)
    w2b = np.asarray(W2, np.float32).astype(BF)
    iota = np.tile(np.arange(128, dtype=np.float32), (128, 1)).astype(BF)
    has_b1 = bool(np.any(np.asarray(b1)))
    has_b2 = bool(np.any(np.asarray(b2)))
    in_maps = []
    for c in range(cfg.r):
        m = {"xT": xT, "W1": w1b, "W2": w2b, "iota": iota}
        m.update(per_core[c])
        if has_b1:
            m["b1b"] = np.tile(np.asarray(b1, np.float32), (128, 1))
        if has_b2:
            m["b2b"] = np.tile(np.asarray(b2, np.float32), (128, 1))
        in_maps.append(m)
    return in_maps, has_b1, has_b2


def kernel(x, edge_index, W1, b1, W2, b2):
    cfg = Cfg()
    nch, per_core, dinv = preprocess(edge_index, cfg)
    in_maps, has_b1, has_b2 = make_in_maps(cfg, per_core, x, dinv,
                                           W1, b1, W2, b2)
    nc = build_program(cfg, nch, has_b1, has_b2)
    res = run_bass_kernel_spmd(nc, in_maps, list(range(cfg.r)))
    return np.concatenate([res.results[c]["out"] for c in range(cfg.r)],
                          axis=0)
